# revision 39
# baseline (speedup 1.0000x reference)
"""Trainium2 Bass kernel for nn_Attention (B=8, N=1024, D=768, H=12).

Strategy: pure data-parallel over batch - core b computes the full attention
layer for batch element b. No collectives.

Schedule (evolved from the 382us baseline, ~325-330us measured):
  - x loaded once in bf16; LN stats via ones-matmuls; rstd = exp(-.5*ln(var))
    on ScalarE (vector.reciprocal is ~6.4ns/elem and bass blocks Rsqrt);
    mean-subtraction explicit, so no extra contraction rows anywhere.
  - scores row-tiled: head pair (2h, 2h+1) runs as two concurrent K=64
    matmuls in PE row-groups (0,0)/(64,0), one [128,2048] PSUM tile per
    (pair, kt); ScalarE exp straight from PSUM into a bf16 [128,8,2048]
    "at" tile; DVE multiplies by host-precomputed exp(bias) (fp8 fails the
    2e-2 budget: e4m3 elementwise error ~5 percent passes through 1:1).
  - emission is a software pipeline: after each score group exactly one
    deferred unit (AV chunk of the previous pair / normalize / qk pack /
    v tile) is emitted so the in-order PE stream always has work while
    ScalarE exps (also keeps the HAM clock-gate from re-throttling).
  - AV deferred by one pair (at ring bufs=2); denominators ride the AV
    matmul as v-column 65, staged to 32-aligned partitions, one batched
    DVE reciprocal per pair, K=1 broadcast matmuls at explicit row groups,
    in-place avT normalize. qkv biases ride the PSUM drains; the v bias
    folds through the softmax (sum attn*(v+c) = av + c*den) into the
    out-projection bias.
  - unit PSUM accumulators are 1-bank [128,512] chunks, double-buffered,
    so each drain overlaps the next chunk's matmuls.
"""

import json
import os
import sys

sys.path.insert(0, "/opt/trn_rl_repo")

import numpy as np
import ml_dtypes

bf16 = ml_dtypes.bfloat16

B, N, D = 8, 1024, 768
H, DH = 12, 64
KT = D // 128          # 6 k-tiles over the model dim
NT = N // 128          # 8 tiles over seq
NCH = N // 512         # 2 free-dim chunks of 512
NP = H // 2            # 6 head pairs
F32 = np.float32

_cache = {}


# ---------------------------------------------------------------------------
# Workaround: this walrus build rejects >1 sync wait per instruction. Split
# excess waits onto same-engine NoOps inserted just before the instruction
# (in-order per engine, so semantics are unchanged).
# ---------------------------------------------------------------------------
def _install_ntff_hook():
    """Provide antenv.axon_hooks if the image lacks it, so trace=True /
    BASS_TRACE=1 can capture NTFF profiles via libaxon_pjrt.so."""
    import types
    import contextlib
    import ctypes

    try:
        import antenv.axon_hooks  # noqa: F401
        return
    except ImportError:
        pass
    import antenv

    mod = types.ModuleType("antenv.axon_hooks")
    holder = [None]
    mod.set_axon_ntff_profile_hook = lambda h: holder.__setitem__(0, h)
    mod.get_axon_ntff_profile_hook = lambda: holder[0]
    sys.modules["antenv.axon_hooks"] = mod
    antenv.axon_hooks = mod

    so_path = "/opt/axon/libaxon_pjrt.so"
    if not os.path.exists(so_path):
        return
    lib = ctypes.CDLL(so_path)
    if not hasattr(lib, "axon_start_nrt_profile"):
        return
    lib.axon_start_nrt_profile.argtypes = [
        ctypes.POINTER(ctypes.c_int64), ctypes.c_size_t]
    lib.axon_start_nrt_profile.restype = ctypes.c_int64
    lib.axon_stop_nrt_profile.argtypes = [ctypes.c_char_p]
    lib.axon_stop_nrt_profile.restype = ctypes.c_int64

    @contextlib.contextmanager
    def _hook(output_dir, device_ids):
        import jax
        jax.devices()
        if device_ids:
            ids = (ctypes.c_int64 * len(device_ids))(*device_ids)
            rc = lib.axon_start_nrt_profile(ids, len(device_ids))
        else:
            rc = lib.axon_start_nrt_profile(None, 0)
        if rc != 0:
            raise RuntimeError(f"axon_start_nrt_profile rc={rc}")
        try:
            yield
        finally:
            n = lib.axon_stop_nrt_profile(str(output_dir).encode())
            print(f"ntff profile: {n} file(s) written to {output_dir}")

    mod.set_axon_ntff_profile_hook(_hook)


def _install_wait_split():
    import concourse.bass_utils as bass_utils
    import concourse.bass2jax as bass2jax

    if getattr(bass_utils, "_wait_split_installed", False):
        return
    orig = bass_utils.compile_bir_kernel
    ctr = [0]

    def _dedup_ldw(d) -> bool:
        """Drop PE Ldweights that reload the exact stationary already in
        the array (walrus --enable-ldw-opt rejects our small-K matmuls, so
        do the elision here). Conservative: only waitless, update-free
        duplicates with an identical ins signature, with nothing but
        Matmults in between."""
        changed = False
        for fn in d.get("functions", []):
            for bb_ in fn.get("blocks", []):
                last_sig = None
                new = []
                for inst in bb_.get("instructions", []):
                    if inst.get("engine") == "PE":
                        op = inst.get("opcode")
                        if op == "Ldweights":
                            si = inst.get("sync_info") or {}
                            sig = (json.dumps(inst.get("ins"),
                                              sort_keys=True) +
                                   str(inst.get("tile_position")))
                            if (sig == last_sig and not si.get("on_wait")
                                    and not si.get("on_update")):
                                changed = True
                                continue
                            last_sig = sig
                        elif op != "Matmult":
                            last_sig = None
                    new.append(inst)
                bb_["instructions"] = new
        return changed

    def _split(bir_json: bytes) -> bytes:
        d = json.loads(bir_json)
        changed = False
        if os.environ.get("LDW_DEDUP", "0") == "1":
            changed = _dedup_ldw(d) or changed
        for fn in d.get("functions", []):
            for bb_ in fn.get("blocks", []):
                new = []
                for inst in bb_.get("instructions", []):
                    si = inst.get("sync_info") or {}
                    ow = si.get("on_wait") or []
                    if len(ow) > 1:
                        changed = True
                        for w in ow[:-1]:
                            ctr[0] += 1
                            new.append({
                                "debug": inst.get("debug", 0),
                                "engine": inst["engine"],
                                "ins": [],
                                "name": f"WSPLIT-{ctr[0]}",
                                "opcode": "NoOp",
                                "outs": [],
                                "sync_info": {"on_update": [], "on_wait": [w]},
                            })
                        si["on_wait"] = [ow[-1]]
                    new.append(inst)
                bb_["instructions"] = new
        return json.dumps(d).encode() if changed else bir_json

    def patched(bir_json, tmpdir, neff_name="file.neff"):
        return orig(_split(bir_json), tmpdir, neff_name=neff_name)

    bass_utils.compile_bir_kernel = patched
    bass2jax.compile_bir_kernel = patched

    # let walrus drop redundant LDWEIGHTS for repeated stationary operands
    if os.environ.get("LDW_OPT", "0") == "1":
        orig_run = bass_utils.run_command

        def run2(cmd, **kw):
            cmd = ["--enable-ldw-opt=true" if c == "--enable-ldw-opt=false"
                   else c for c in cmd]
            return orig_run(cmd, **kw)

        bass_utils.run_command = run2
    bass_utils._wait_split_installed = True


# ---------------------------------------------------------------------------
# Builder
# ---------------------------------------------------------------------------
def _build():
    import contextlib

    import concourse.bass as bass
    import concourse.tile as tile
    from concourse import mybir

    dt = mybir.dt
    AF = mybir.ActivationFunctionType

    nc = bass.Bass("TRN2", target_bir_lowering=False, debug=False)

    xT = nc.declare_dram_parameter("xT", [D, N], dt.bfloat16, isOutput=False)
    # q/k weights fp8 e4m3 (x64-scaled, DoubleRow-interleaved): score
    # errors wash out in the softmax average. v/wo stay bf16 (their errors
    # hit the output directly). LN mean-subtraction is explicit (no extras
    # rows); qkv biases ride the PSUM drains, v-bias folds into the out
    # bias through the softmax identity sum(attn*(v+c)) = av + c*den.
    wq = nc.declare_dram_parameter("wq", [D, D], dt.bfloat16, isOutput=False)
    wk = nc.declare_dram_parameter("wk", [D, D], dt.bfloat16, isOutput=False)
    wv = nc.declare_dram_parameter("wv", [D, D], dt.bfloat16, isOutput=False)
    wo = nc.declare_dram_parameter("wo", [D, D], dt.bfloat16, isOutput=False)
    cbp = nc.declare_dram_parameter("cbp", [128, 2 * KT], dt.float32,
                                    isOutput=False)
    bop = nc.declare_dram_parameter("bop", [128, KT], dt.float32,
                                    isOutput=False)
    expb = nc.declare_dram_parameter("expb", [H, N, N], dt.bfloat16, isOutput=False)
    outT = nc.declare_dram_parameter("outT", [D, N], dt.float32, isOutput=True)

    with tile.TileContext(nc) as tc:
        ctx = contextlib.ExitStack()
        with ctx:
            # ---- SBUF pools ----
            sing = ctx.enter_context(tc.tile_pool(name="sing", bufs=1))
            wp = ctx.enter_context(tc.tile_pool(name="wp", bufs=1))
            actp = ctx.enter_context(tc.tile_pool(name="actp", bufs=1))
            ebp = ctx.enter_context(tc.tile_pool(name="ebp", bufs=2))
            # xt tiles freed after the xs pass
            xt_ctx = contextlib.ExitStack()
            xtp = xt_ctx.enter_context(tc.tile_pool(name="xtp", bufs=1))
            sqp = xt_ctx.enter_context(tc.tile_pool(name="sqp", bufs=1))
            stp = xt_ctx.enter_context(tc.tile_pool(name="stp", bufs=1))
            # ---- PSUM pools ----
            # big: qk/v accumulators ([128,1024] = 2 banks), whole kernel
            pbig = ctx.enter_context(tc.tile_pool(name="pbig", bufs=2,
                                                  space="PSUM"))
            # stats psums (6 banks), freed before the head loop opens psc/pa
            st_ctx = contextlib.ExitStack()
            pst = st_ctx.enter_context(tc.tile_pool(name="pst", bufs=1,
                                                    space="PSUM"))

            # --- constants ---
            ones_col_b = sing.tile([128, 1], dt.bfloat16, tag="ones_col_b")
            nc.gpsimd.memset(ones_col_b[:], 1.0)
            ones_row = sing.tile([1, 128], dt.bfloat16, tag="ones_row")
            nc.gpsimd.memset(ones_row[:], 1.0)
            ones128b = sing.tile([128, 64], dt.bfloat16, tag="ones128b")
            nc.gpsimd.memset(ones128b[:], 1.0)
            eps_t = sing.tile([1, 1], dt.float32, tag="eps")
            nc.gpsimd.memset(eps_t[:], 1e-5)
            dstage = sing.tile([128, 512], dt.float32, tag="dstage")
            nc.gpsimd.memset(dstage[:], 1.0)
            rcp_s = sing.tile([128, 512], dt.bfloat16, tag="rcp_s")
            # touch Ln/Exp right away so the ACT table set loads during
            # the initial DMAs instead of on the LN critical path
            warmup = sing.tile([1, 1], dt.float32, tag="warmup")
            nc.scalar.activation(warmup[:], eps_t[:], AF.Ln)
            nc.scalar.activation(warmup[:], warmup[:], AF.Exp)

            # --- eb prefetch (biggest DMA stream; lo half on the gpsimd
            # queue right away, hi half on sync after the weight loads) ---
            eb_t = {}

            def issue_eb_lo(h):
                t = ebp.tile([128, NT, N], dt.bfloat16, tag="eb")
                src = expb[h].rearrange("(kt p) q -> p kt q", p=128)
                nc.gpsimd.dma_start(out=t[:, 0:4, :], in_=src[:, 0:4, :])
                eb_t[h] = t

            def issue_eb_hi(h):
                src = expb[h].rearrange("(kt p) q -> p kt q", p=128)
                nc.sync.dma_start(out=eb_t[h][:, 4:8, :], in_=src[:, 4:8, :])

            def issue_eb(h):
                issue_eb_lo(h)
                issue_eb_hi(h)

            issue_eb_lo(0)
            issue_eb_lo(1)

            # --- x tiles (bf16, single load) + weights ---
            xt = []
            for i in range(KT):
                t = xtp.tile([128, N], dt.bfloat16, tag=f"xt{i}")
                eng = nc.sync if i % 2 == 0 else nc.gpsimd
                eng.dma_start(out=t[:], in_=xT[i * 128:(i + 1) * 128, :])
                xt.append(t)

            def load_w(name, par):
                ts_ = []
                for t in range(KT):
                    w = wp.tile([128, D], dt.bfloat16, tag=f"{name}{t}")
                    nc.sync.dma_start(out=w[:], in_=par[t * 128:(t + 1) * 128, :])
                    ts_.append(w)
                return ts_

            wqt = load_w("wq", wq)
            wkt = load_w("wk", wk)
            wvt = load_w("wv", wv)
            wot = load_w("wo", wo)
            cbp_t = wp.tile([128, 2 * KT], dt.float32, tag="cbp")
            nc.sync.dma_start(out=cbp_t[:], in_=cbp[:, :])
            bop_t = wp.tile([128, KT], dt.float32, tag="bop")
            nc.sync.dma_start(out=bop_t[:], in_=bop[:, :])
            issue_eb_hi(0)
            issue_eb_hi(1)

            # --- pass 1: LN stats from bf16 x via ones-matmuls ---
            psum = pst.tile([1, N], dt.float32, tag="psum")
            psq = pst.tile([1, N], dt.float32, tag="psq")
            for i in range(KT):
                sq = sqp.tile([128, N], dt.bfloat16, tag="sq")
                nc.vector.tensor_mul(sq[:], xt[i][:], xt[i][:])
                for c in range(NCH):
                    cs = slice(c * 512, (c + 1) * 512)
                    nc.tensor.matmul(psum[:, cs], ones_col_b[:], xt[i][:, cs],
                                     start=(i == 0), stop=(i == KT - 1))
                    nc.tensor.matmul(psq[:, cs], ones_col_b[:], sq[:, cs],
                                     start=(i == 0), stop=(i == KT - 1))

            # 3 reusable [1,N] f32 scratch rows (each costs 4KB/partition)
            sa = stp.tile([1, N], dt.float32, tag="sa")   # mu
            sb = stp.tile([1, N], dt.float32, tag="sb")   # msq->var->rstd
            sc = stp.tile([1, N], dt.float32, tag="sc2")  # mu^2->lnv->mrs
            nc.vector.tensor_scalar_mul(sa[:], psum[:], 1.0 / D)
            nc.vector.tensor_scalar_mul(sb[:], psq[:], 1.0 / D)
            nc.vector.tensor_mul(sc[:], sa[:], sa[:])
            nc.vector.tensor_sub(sb[:], sb[:], sc[:])      # var
            # rstd = exp(-0.5*ln(var+eps)): two fast ScalarE table ops
            # instead of the slow DVE reciprocal on the critical path
            nc.scalar.activation(sc[:], sb[:], AF.Ln, bias=eps_t[:])
            rstd = stp.tile([1, N], dt.bfloat16, tag="rstdb")
            with nc.allow_low_precision(reason="rstd bf16 fine at 2e-2"):
                nc.scalar.activation(rstd[:], sc[:], AF.Exp, scale=-0.5)
            mu16 = stp.tile([1, N], dt.bfloat16, tag="mu16")
            nc.vector.tensor_copy(mu16[:], sa[:])
            # broadcast rstd and mu to all 128 partitions (K=1 fp32
            # matmuls, reusing one PSUM slab)
            prb = pst.tile([128, N], dt.float32, tag="prb")
            for c in range(NCH):
                cs = slice(c * 512, (c + 1) * 512)
                nc.tensor.matmul(prb[:, cs], ones_row[:], rstd[:, cs],
                                 start=True, stop=True)
            rstd_b = sing.tile([128, N], dt.bfloat16, tag="rstd_b")
            nc.scalar.copy(rstd_b[:], prb[:])
            prb2 = pst.tile([128, N], dt.float32, tag="prb")
            for c in range(NCH):
                cs = slice(c * 512, (c + 1) * 512)
                nc.tensor.matmul(prb2[:, cs], ones_row[:], mu16[:, cs],
                                 start=True, stop=True)
            mu_b = sing.tile([128, N], dt.bfloat16, tag="mu_b")
            nc.scalar.copy(mu_b[:], prb2[:])

            # --- pass 2: xs = (x - mu) * rstd in bf16 (for v) and an fp8
            # copy in one [128, KT, N] tile so DoubleRow q/k matmuls can
            # slice k-tile pairs [128, 2, N] ---
            xs_bf = actp.tile([128, KT, N], dt.bfloat16, tag="xsbf")
            for i in range(KT):
                nc.vector.tensor_sub(xs_bf[:, i, :], xt[i][:], mu_b[:])
                nc.vector.tensor_mul(xs_bf[:, i, :], xs_bf[:, i, :],
                                     rstd_b[:])

            xt_ctx.close()
            st_ctx.close()
            # pools created after the xt/stats space is freed (baseline
            # pattern: allocator reuses closed-pool space for later pools)
            qkp = ctx.enter_context(tc.tile_pool(name="qkp", bufs=6))
            atp = ctx.enter_context(tc.tile_pool(name="atp", bufs=2))
            outp = ctx.enter_context(tc.tile_pool(name="outp", bufs=1))
            # head-phase PSUM pools: scores 4 banks + pa 2 banks (+big 2 = 8)
            hd_ctx = contextlib.ExitStack()
            psc = hd_ctx.enter_context(tc.tile_pool(name="psc", bufs=1,
                                                    space="PSUM"))
            pa = hd_ctx.enter_context(tc.tile_pool(name="pa", bufs=2,
                                                   space="PSUM"))

            # --- qT/kT pack projection: one "unit" = one 128-row pack.
            # fp8 DoubleRow over 3 double-k-tiles + bf16 K=2 extras matmul
            # into the same fp32 PSUM group; 1/64 descale on the drain. ---
            qT = [None] * KT
            kT = [None] * KT

            def qk_unit(wts, cb_j, dest, name, p):
                """One pack, processed as two 1-bank psum chunks so each
                chunk's drain overlaps the next chunk's matmuls (pbig is
                double-buffered)."""
                t = qkp.tile([128, N], dt.bfloat16, tag="qkT",
                             name=f"{name}{p}")
                pc = slice(p * 128, (p + 1) * 128)
                for c in range(NCH):
                    cs = slice(c * 512, (c + 1) * 512)
                    pq = pbig.tile([128, 512], dt.float32, tag="big",
                                   name=f"pq_{name}{p}_{c}")
                    for kt_ in range(KT):
                        nc.tensor.matmul(pq[:], wts[kt_][:, pc],
                                         xs_bf[:, kt_, cs],
                                         start=(kt_ == 0),
                                         stop=(kt_ == KT - 1))
                    # drain: add the projection bias column (ScalarE,
                    # freeing DVE)
                    nc.scalar.activation(t[:, cs], pq[:], AF.Identity,
                                         bias=cbp_t[:, cb_j:cb_j + 1])
                dest[p] = t

            # --- v unit: activations stationary, [seq, head, 64+ones] ---
            v_ext = [None] * NT

            def v_unit(s):
                vt = actp.tile([128, H, 65], dt.bfloat16, tag=f"v{s}")
                nc.gpsimd.memset(vt[:, :, 64:65], 1.0)
                ss = slice(s * 128, (s + 1) * 128)
                for c0, cw, h0, h1 in [(0, 512, 0, 8), (512, 256, 8, 12)]:
                    pv = pbig.tile([128, 512], dt.float32, tag="big",
                                   name=f"pv{s}_{h0}")
                    for kt_ in range(KT):
                        nc.tensor.matmul(pv[:, 0:cw], xs_bf[:, kt_, ss],
                                         wvt[kt_][:, c0:c0 + cw],
                                         start=(kt_ == 0),
                                         stop=(kt_ == KT - 1))
                    nc.vector.tensor_copy(
                        vt[:, h0:h1, 0:64],
                        pv[:, 0:cw].rearrange("p (h c) -> p h c", c=64))
                v_ext[s] = vt

            # prologue packs: pair 0 needs its q/k before scoring
            qk_unit(wqt, 0, qT, "qT", 0)
            qk_unit(wkt, KT + 0, kT, "kT", 0)

            # avT accumulators (2 heads per tile, unnormalized until norm_q)
            avT = [actp.tile([128, N], dt.bfloat16, tag=f"avT{p}",
                             name=f"avT{p}") for p in range(NP)]
            at_t = {}

            def emit_ebmul(p, at, k0, k1):
                # split so the last kt's multiply (the AV-gating one) is
                # small and lands right after its exp
                gs = slice(k0, k1)
                for h2 in range(2):
                    h = 2 * p + h2
                    nc.vector.tensor_mul(
                        at[:, gs, h2 * 1024:(h2 + 1) * 1024],
                        at[:, gs, h2 * 1024:(h2 + 1) * 1024],
                        eb_t[h][:, gs, :])

            def av_unit(p, h2, c, last=False):
                """AV for one (head, chunk): ~1.8us of contiguous PE work.
                Denominator rides col 64 (M=65), staged to a 32-aligned
                partition; the pair's batched reciprocal rides the last
                unit."""
                at = at_t[p]
                h = 2 * p + h2
                rs2 = slice(64 * h2, 64 * h2 + 64)
                cs = slice(c * 512, (c + 1) * 512)
                pav = pa.tile([65, 512], dt.float32, tag="pa")
                for kt_ in range(NT):
                    nc.tensor.matmul(
                        pav[:], v_ext[kt_][:, h, :],
                        at[:, kt_, h2 * 1024 + c * 512:
                           h2 * 1024 + (c + 1) * 512],
                        start=(kt_ == 0), stop=(kt_ == NT - 1))
                nc.scalar.copy(avT[p][rs2, cs], pav[0:64, :])
                j = 32 * (2 * h2 + c)
                nc.scalar.copy(dstage[j:j + 1, :], pav[64:65, :])
                if last:
                    with nc.allow_low_precision(
                            reason="softmax denominators in bf16 are fine"):
                        nc.vector.reciprocal(rcp_s[:], dstage[:])

            def norm_unit(p):
                """Normalize avT[p] rows using the pair's staged reciprocals
                via K=1 broadcast matmuls at explicit row-groups."""
                for h2 in range(2):
                    rs2 = slice(64 * h2, 64 * h2 + 64)
                    for c in range(NCH):
                        cs = slice(c * 512, (c + 1) * 512)
                        j = 32 * (2 * h2 + c)
                        pbc = pa.tile([64, 512], dt.float32, tag="pa")
                        nc.tensor.matmul(pbc[:], ones128b[j:j + 1, :],
                                         rcp_s[j:j + 1, :],
                                         start=True, stop=True,
                                         tile_position=(j, 0))
                        nc.vector.tensor_mul(avT[p][rs2, cs],
                                             avT[p][rs2, cs], pbc[:])

            def emit_scores(p, at, queue):
                """Row-tiled scores + exp for pair p; exactly one deferred
                work unit (AV/norm/qk/v) is emitted after each score group
                so the PE stream stays contiguous while ScalarE exps."""
                pops = [0] * NT
                for i in range(len(queue)):
                    pops[min(NT - 1,
                             int((i + 1.0) * NT / (len(queue) + 1)))] += 1
                for kt_ in range(NT):
                    pt = psc.tile([128, 2048], dt.float32, tag="sc")
                    ks = slice(kt_ * 128, (kt_ + 1) * 128)
                    for h2 in range(2):
                        rs2 = slice(64 * h2, 64 * h2 + 64)
                        for c in range(NCH):
                            nc.tensor.matmul(
                                pt[:, h2 * 1024 + c * 512:
                                   h2 * 1024 + (c + 1) * 512],
                                kT[p][rs2, ks],
                                qT[p][rs2, c * 512:(c + 1) * 512],
                                start=True, stop=True)
                    nc.scalar.activation(at[:, kt_, :], pt[:], AF.Exp)
                    for _ in range(pops[kt_]):
                        if queue:
                            queue.pop(0)()
                    if kt_ == 2:
                        emit_ebmul(p, at, 0, 3)
                    elif kt_ == 3:
                        emit_ebmul(p, at, 3, 4)
                    elif kt_ == 6:
                        emit_ebmul(p, at, 4, 7)
                emit_ebmul(p, at, 7, 8)
                while queue:
                    queue.pop(0)()

            def av_units(p):
                return [
                    (lambda h2=h2, c=c: av_unit(p, h2, c,
                                                last=(h2 == 1 and c == 1)))
                    for h2 in range(2) for c in range(NCH)]

            # per-pair deferred-work queues (see docstring of emit_scores)
            queues = {
                0: [lambda: qk_unit(wqt, 1, qT, "qT", 1),
                    lambda: qk_unit(wkt, KT + 1, kT, "kT", 1)] +
                   [lambda s=s: v_unit(s) for s in range(6)],
                1: [lambda: v_unit(6), lambda: v_unit(7)] + av_units(0) +
                   [lambda: qk_unit(wqt, 2, qT, "qT", 2),
                    lambda: qk_unit(wkt, KT + 2, kT, "kT", 2)],
            }
            for p in range(2, NP):
                # AV units early: the at-ring slot they read is the one the
                # NEXT pair's first exp overwrites, so freeing it early cuts
                # the pair-boundary stall. norm(p-2) must still pop BEFORE
                # the recip-carrying last AV unit (rcp_s lifetime).
                avs = av_units(p - 1)
                q = avs[0:3] + [lambda p=p: norm_unit(p - 2)] + avs[3:4]
                if p + 1 < KT:
                    q += [lambda p=p: qk_unit(wqt, p + 1, qT, "qT", p + 1),
                          lambda p=p: qk_unit(wkt, KT + p + 1, kT,
                                              "kT", p + 1)]
                queues[p] = q

            # --- head-pair loop ---
            for p in range(NP):
                if p + 1 < NP:
                    issue_eb(2 * (p + 1))
                    issue_eb(2 * (p + 1) + 1)
                at = atp.tile([128, NT, 2048], dt.bfloat16, tag="at")
                at_t[p] = at
                emit_scores(p, at, queues[p])
            # order matters: norm(p) must read rcp_s before av(p+1)'s
            # reciprocal overwrites it
            norm_unit(NP - 2)
            for u in av_units(NP - 1):
                u()
            norm_unit(NP - 1)

            hd_ctx.close()
            pout = ctx.enter_context(tc.tile_pool(name="pout", bufs=2,
                                                  space="PSUM"))

            # --- output projection (bias folded; transposed out) ---
            for mt in range(KT):
                mc = slice(mt * 128, (mt + 1) * 128)
                py = pout.tile([128, N], dt.float32, tag="py")
                for kt_ in range(KT):
                    for c in range(NCH):
                        cs = slice(c * 512, (c + 1) * 512)
                        nc.tensor.matmul(py[:, cs], wot[kt_][:, mc],
                                         avT[kt_][:, cs],
                                         start=(kt_ == 0),
                                         stop=(kt_ == KT - 1))
                ot = outp.tile([128, N], dt.float32, tag="ot")
                # bias_eff = b_out + Wo.T @ cb_v rides the drain
                nc.scalar.activation(ot[:], py[:], AF.Identity,
                                     bias=bop_t[:, mt:mt + 1])
                (nc.sync if mt % 2 == 0 else nc.gpsimd).dma_start(
                    out=outT[mc, :], in_=ot[:])

    return nc


# ---------------------------------------------------------------------------
# Host side
# ---------------------------------------------------------------------------
def _host_prep(x, rpb, W_qkv, W_out, b_out, ln_g, ln_b):
    g = np.asarray(ln_g, F32)
    bb_ = np.asarray(ln_b, F32)
    W_qkv = np.asarray(W_qkv, F32)
    W_out = np.asarray(W_out, F32)
    b_out = np.asarray(b_out, F32)

    def make_w(W, scale=1.0):
        Wp = (g[:, None] * W) * scale
        cb = (bb_[:, None] * W).sum(axis=0) * scale
        return np.ascontiguousarray(Wp.astype(bf16)), cb

    wq_, cbq = make_w(W_qkv[:, :D], 1.0 / np.sqrt(DH))
    wk_, cbk = make_w(W_qkv[:, D:2 * D])
    wv_, cbv = make_w(W_qkv[:, 2 * D:])
    wo = np.ascontiguousarray(W_out.astype(bf16))
    # v bias folds through the softmax into the out-proj bias
    bias_eff = b_out + W_out.T @ cbv
    cbp = np.ascontiguousarray(
        np.concatenate([cbq, cbk]).reshape(2 * KT, 128).T.astype(F32))
    bop = np.ascontiguousarray(bias_eff.reshape(KT, 128).T.astype(F32))
    expb = np.ascontiguousarray(
        np.exp(np.asarray(rpb, F32)[0].transpose(0, 2, 1)).astype(bf16))

    shared = {"wq": wq_, "wk": wk_, "wv": wv_, "wo": wo,
              "cbp": cbp, "bop": bop, "expb": expb}
    in_maps = []
    for b_i in range(B):
        m = dict(shared)
        m["xT"] = np.ascontiguousarray(np.asarray(x[b_i], F32).T.astype(bf16))
        in_maps.append(m)
    return in_maps


def kernel(x, relative_position_bias, W_qkv, W_out, b_out, ln_g, ln_b):
    _install_wait_split()
    _install_ntff_hook()
    from concourse.bass_utils import run_bass_kernel_spmd

    if "nc" not in _cache:
        _cache["nc"] = _build()
    nc = _cache["nc"]

    in_maps = _host_prep(x, relative_position_bias, W_qkv, W_out, b_out,
                         ln_g, ln_b)
    res = run_bass_kernel_spmd(nc, in_maps, core_ids=list(range(B)))
    _cache["last_result"] = res

    out = np.empty((B, N, D), F32)
    for b_i in range(B):
        out[b_i] = res.results[b_i]["outT"].T
    return out
